# revision 8
# baseline (speedup 1.0000x reference)
"""Trainium2 Bass kernel for nn_ContrastiveLoss (InfoNCE-style loss).

Sharding: data-parallel over nodes N=200000 across 8 NeuronCores
(25000 nodes/core, padded to 25088 = 128 partitions x 196 tile slots).
Node j of a shard maps partition-major: j = p*196 + t, so the shard's
e-rows load as one fully-contiguous HWDGE DMA per core.

Device pipeline per group of T_PER tiles:
  - ONE batched indirect-DMA gather of T_PER*11 rows/partition from a
    combined [pos;neg] bf16 table (row indices precomputed on host)
  - DVE bf16 multiply (2x perf mode) against the broadcast e-row
  - halving-tree adds in bf16 (+final fp32 tensor_reduce) -> dots
Epilogue (batched over all 196 tiles): rowmax, subtract, ACT Exp,
segmented sum, ACT Ln, score = (m - dots0)/tau + ln(sum).
Host masks padding and takes the global mean over the 8 cores.
"""

import numpy as np

TAU = 0.65
NUM_NEG = 10
S_ALL = NUM_NEG + 1             # pos + 10 negatives
N, M, D = 200000, 200000, 128
N_CORES = 8
N_SHARD = N // N_CORES          # 25000
TILES = 196                     # ceil(25000/128)
N_PAD = TILES * 128             # 25088
T_PER = 2                       # tiles per DVE compute group
BAR_EVERY_G = 2                 # groups between scheduler fences

_COMPILED = None
_DONATE_OUTPUTS = True          # sim_check disables (CPU can't alias)


def _build_nc():
    import concourse.bass as bass
    import concourse.bacc as bacc
    import concourse.tile as tile
    from concourse import mybir

    F32 = mybir.dt.float32
    BF16 = mybir.dt.bfloat16
    I32 = mybir.dt.int32
    AF = mybir.ActivationFunctionType
    OP = mybir.AluOpType
    AX = mybir.AxisListType

    nc = bacc.Bacc("TRN2", target_bir_lowering=False, debug=False,
                   num_devices=N_CORES)
    comb = nc.dram_tensor("comb", [N + M, D], BF16, kind="ExternalInput").ap()
    esh = nc.dram_tensor("esh", [N_PAD, D], BF16, kind="ExternalInput").ap()
    idx = nc.dram_tensor("idx", [128, TILES, S_ALL], I32,
                         kind="ExternalInput").ap()
    scores_out = nc.dram_tensor("scores", [128, TILES], F32,
                                kind="ExternalOutput").ap()

    G = TILES // T_PER
    with tile.TileContext(nc) as tc:
        with tc.tile_pool(name="consts", bufs=1) as consts, \
             tc.tile_pool(name="xs", bufs=4) as xs, \
             tc.tile_pool(name="pr", bufs=2) as pr, \
             tc.tile_pool(name="tr", bufs=2) as tr, \
             tc.tile_pool(name="ep", bufs=1) as ep:
            idx_sb = consts.tile([128, TILES, S_ALL], I32, tag="idx")
            nc.sync.dma_start(out=idx_sb[:], in_=idx[:])
            ebf = consts.tile([128, TILES, D], BF16, tag="ebf")
            nc.sync.dma_start(
                out=ebf[:], in_=esh.rearrange("(p t) d -> p t d", p=128))
            dots = ep.tile([128, TILES, S_ALL], F32, tag="dots")

            for g in range(G):
                t0 = g * T_PER
                samp = xs.tile([128, T_PER * S_ALL, D], BF16, tag="s")
                # HW vector-indirect DMA supports exactly one offset per
                # partition, so issue one gather per (tile, sample).
                for ti in range(T_PER):
                    for s in range(S_ALL):
                        nc.gpsimd.indirect_dma_start(
                            out=samp[:, ti * S_ALL + s, :],
                            out_offset=None,
                            in_=comb[:, :],
                            in_offset=bass.IndirectOffsetOnAxis(
                                ap=idx_sb[:, t0 + ti, s:s + 1], axis=0),
                        )
                prod = pr.tile([128, T_PER, S_ALL, D], BF16, tag="p")
                nc.vector.tensor_tensor(
                    out=prod[:],
                    in0=samp[:].rearrange("p (t s) d -> p t s d", t=T_PER),
                    in1=ebf[:, t0:t0 + T_PER, :]
                        .rearrange("p t (o d) -> p t o d", o=1)
                        .to_broadcast([128, T_PER, S_ALL, D]),
                    op=OP.mult)
                h = tr.tile([128, T_PER, S_ALL, 64], BF16, tag="h")
                nc.vector.tensor_tensor(
                    out=h[:], in0=prod[:, :, :, 0:64],
                    in1=prod[:, :, :, 64:128], op=OP.add)
                q = tr.tile([128, T_PER, S_ALL, 32], BF16, tag="q")
                nc.vector.tensor_tensor(
                    out=q[:], in0=h[:, :, :, 0:32], in1=h[:, :, :, 32:64],
                    op=OP.add)
                e8 = tr.tile([128, T_PER, S_ALL, 16], BF16, tag="e8")
                nc.vector.tensor_tensor(
                    out=e8[:], in0=q[:, :, :, 0:16], in1=q[:, :, :, 16:32],
                    op=OP.add)
                nc.vector.tensor_reduce(
                    out=dots[:, t0:t0 + T_PER, :], in_=e8[:], axis=AX.X,
                    op=OP.add)
                if (g + 1) % BAR_EVERY_G == 0:
                    tc.no_sync_barrier()

            # batched logsumexp epilogue over all tiles
            mx = ep.tile([128, TILES], F32, tag="mx")
            nc.vector.tensor_reduce(out=mx[:], in_=dots[:], axis=AX.X,
                                    op=OP.max)
            subx = ep.tile([128, TILES, S_ALL], F32, tag="subx")
            nc.vector.tensor_tensor(
                out=subx[:], in0=dots[:],
                in1=mx[:].rearrange("p (t o) -> p t o", o=1)
                    .to_broadcast([128, TILES, S_ALL]),
                op=OP.subtract)
            expd = ep.tile([128, TILES, S_ALL], F32, tag="expd")
            nc.scalar.activation(out=expd[:], in_=subx[:], func=AF.Exp,
                                 scale=1.0 / TAU)
            se = ep.tile([128, TILES], F32, tag="se")
            nc.vector.tensor_reduce(out=se[:], in_=expd[:], axis=AX.X,
                                    op=OP.add)
            lnv = ep.tile([128, TILES], F32, tag="lnv")
            nc.scalar.activation(out=lnv[:], in_=se[:], func=AF.Ln)
            t1 = ep.tile([128, TILES], F32, tag="t1")
            nc.vector.tensor_tensor(out=t1[:], in0=mx[:], in1=dots[:, :, 0],
                                    op=OP.subtract)
            scores_sb = ep.tile([128, TILES], F32, tag="scores")
            nc.vector.scalar_tensor_tensor(
                out=scores_sb[:], in0=t1[:], scalar=1.0 / TAU, in1=lnv[:],
                op0=OP.mult, op1=OP.add)
            nc.sync.dma_start(out=scores_out[:], in_=scores_sb[:])
    nc.compile()
    return nc


class _Runner:
    """Compile-once SPMD runner over axon PJRT (8 cores, shard_map)."""

    def __init__(self, nc):
        import jax
        import numpy as np
        from jax.sharding import Mesh, PartitionSpec
        from jax.experimental.shard_map import shard_map
        from concourse import bass2jax, mybir
        from concourse.bass2jax import _bass_exec_p, install_neuronx_cc_hook

        install_neuronx_cc_hook()
        self.jax = jax
        partition_name = (
            nc.partition_id_tensor.name if nc.partition_id_tensor else None
        )
        in_names, out_names, out_avals, zero_outs = [], [], [], []
        for alloc in nc.m.functions[0].allocations:
            if not isinstance(alloc, mybir.MemoryLocationSet):
                continue
            name = alloc.memorylocations[0].name
            if alloc.kind == "ExternalInput":
                if name != partition_name:
                    in_names.append(name)
            elif alloc.kind == "ExternalOutput":
                shape = tuple(alloc.tensor_shape)
                dtype = mybir.dt.np(alloc.dtype)
                out_names.append(name)
                out_avals.append(jax.core.ShapedArray(shape, dtype))
                zero_outs.append(np.zeros(shape, dtype))
        self.in_names, self.out_names = in_names, out_names
        self.out_avals, self.zero_outs = out_avals, zero_outs
        n_params, n_outs = len(in_names), len(out_names)
        all_in_names = in_names + out_names
        if partition_name is not None:
            all_in_names.append(partition_name)
        donate = (tuple(range(n_params, n_params + n_outs))
                  if _DONATE_OUTPUTS else ())

        def _body(*args):
            operands = list(args)
            if partition_name is not None:
                operands.append(bass2jax.partition_id_tensor())
            outs = _bass_exec_p.bind(
                *operands,
                out_avals=tuple(out_avals),
                in_names=tuple(all_in_names),
                out_names=tuple(out_names),
                lowering_input_output_aliases=(),
                sim_require_finite=True,
                sim_require_nnan=True,
                nc=nc,
            )
            return tuple(outs)

        devices = jax.devices()[:N_CORES]
        self.mesh = Mesh(np.asarray(devices), ("core",))
        in_specs = (PartitionSpec("core"),) * (n_params + n_outs)
        out_specs = (PartitionSpec("core"),) * n_outs
        self.fn = jax.jit(
            shard_map(_body, mesh=self.mesh, in_specs=in_specs,
                      out_specs=out_specs, check_rep=False),
            donate_argnums=donate, keep_unused=True)
        self._staged = None

    def stage(self, in_maps):
        from jax.sharding import NamedSharding, PartitionSpec
        concat = [
            np.ascontiguousarray(
                np.concatenate([np.asarray(m[n]) for m in in_maps], axis=0))
            for n in self.in_names
        ]
        sh = NamedSharding(self.mesh, PartitionSpec("core"))
        self._staged = [self.jax.device_put(a, sh) for a in concat]
        self.jax.block_until_ready(self._staged)

    def run(self):
        zeros = [
            np.zeros((N_CORES * z.shape[0], *z.shape[1:]), z.dtype)
            for z in self.zero_outs
        ]
        out = self.fn(*self._staged, *zeros)
        self.jax.block_until_ready(out)
        return [
            {
                n: np.asarray(out[i]).reshape(
                    N_CORES, *self.out_avals[i].shape)[c]
                for i, n in enumerate(self.out_names)
            }
            for c in range(N_CORES)
        ]


def _prep_inputs(embeddings, negative_embs, community_pos_options, neg_idx,
                 iter_n):
    from concourse import mybir
    bf16 = mybir.dt.np(mybir.dt.bfloat16)
    embeddings = np.asarray(embeddings, dtype=np.float32)
    negative_embs = np.asarray(negative_embs, dtype=np.float32)
    comb = np.ascontiguousarray(
        np.concatenate([embeddings, negative_embs], axis=0).astype(bf16))
    cpo = np.asarray(community_pos_options)
    nidx = np.asarray(neg_idx)
    it = int(np.asarray(iter_n))
    pos_idx = cpo[:, it - 1].astype(np.int32)          # [N] rows of comb
    nidx32 = (nidx.astype(np.int32) + np.int32(N))     # [NUM_NEG, N] rows

    in_maps = []
    for c in range(N_CORES):
        base = c * N_SHARD
        esh = np.zeros((N_PAD, D), bf16)
        esh[:N_SHARD] = embeddings[base:base + N_SHARD].astype(bf16)
        pos_pad = np.zeros(N_PAD, np.int32)
        pos_pad[:N_SHARD] = pos_idx[base:base + N_SHARD]
        neg_pad = np.zeros((NUM_NEG, N_PAD), np.int32)
        neg_pad[:, :N_SHARD] = nidx32[:, base:base + N_SHARD]
        # node j = p*196 + t  ->  idx[p, t, s]
        idx = np.empty((128, TILES, S_ALL), np.int32)
        idx[:, :, 0] = pos_pad.reshape(128, TILES)
        idx[:, :, 1:] = neg_pad.reshape(NUM_NEG, 128, TILES).transpose(1, 2, 0)
        in_maps.append({
            "comb": comb,
            "esh": esh,
            "idx": idx,
        })
    return in_maps


def _get_runner():
    global _COMPILED
    if _COMPILED is None:
        _COMPILED = _Runner(_build_nc())
    return _COMPILED


def kernel(embeddings, negative_embs, community_pos_options, neg_idx, iter_n):
    r = _get_runner()
    in_maps = _prep_inputs(embeddings, negative_embs, community_pos_options,
                           neg_idx, iter_n)
    r.stage(in_maps)
    res = r.run()
    total = 0.0
    for c in range(N_CORES):
        sc = res[c]["scores"]         # [128, TILES], node j at [j//196, j%196]
        flat = sc.reshape(-1)         # node-ordered
        total += float(flat[:N_SHARD].astype(np.float64).sum())
    return np.float32(total / N)


# revision 9
# speedup vs baseline: 1.0144x; 1.0144x over previous
"""Trainium2 Bass kernel for nn_ContrastiveLoss (InfoNCE-style loss).

Sharding: data-parallel over nodes N=200000 across 8 NeuronCores
(25000 nodes/core, padded to 25088 = 128 partitions x 196 tile slots).
Node j of a shard maps partition-major: j = p*196 + t, so the shard's
e-rows load as one fully-contiguous HWDGE DMA per core.

Device pipeline per group of T_PER tiles:
  - ONE batched indirect-DMA gather of T_PER*11 rows/partition from a
    combined [pos;neg] bf16 table (row indices precomputed on host)
  - DVE bf16 multiply (2x perf mode) against the broadcast e-row
  - halving-tree adds in bf16 (+final fp32 tensor_reduce) -> dots
Epilogue (batched over all 196 tiles): rowmax, subtract, ACT Exp,
segmented sum, ACT Ln, score = (m - dots0)/tau + ln(sum).
Host masks padding and takes the global mean over the 8 cores.
"""

import numpy as np

TAU = 0.65
NUM_NEG = 10
S_ALL = NUM_NEG + 1             # pos + 10 negatives
N, M, D = 200000, 200000, 128
N_CORES = 8
N_SHARD = N // N_CORES          # 25000
TILES = 196                     # ceil(25000/128)
N_PAD = TILES * 128             # 25088
T_PER = 2                       # tiles per DVE compute group
BAR_EVERY_G = 14                # groups between scheduler fences

_COMPILED = None
_DONATE_OUTPUTS = True          # sim_check disables (CPU can't alias)


def _build_nc():
    import concourse.bass as bass
    import concourse.bacc as bacc
    import concourse.tile as tile
    from concourse import mybir

    F32 = mybir.dt.float32
    BF16 = mybir.dt.bfloat16
    I32 = mybir.dt.int32
    AF = mybir.ActivationFunctionType
    OP = mybir.AluOpType
    AX = mybir.AxisListType

    nc = bacc.Bacc("TRN2", target_bir_lowering=False, debug=False,
                   num_devices=N_CORES)
    comb = nc.dram_tensor("comb", [N + M, D], BF16, kind="ExternalInput").ap()
    esh = nc.dram_tensor("esh", [N_PAD, D], BF16, kind="ExternalInput").ap()
    idx = nc.dram_tensor("idx", [128, TILES, S_ALL], I32,
                         kind="ExternalInput").ap()
    scores_out = nc.dram_tensor("scores", [128, TILES], F32,
                                kind="ExternalOutput").ap()

    G = TILES // T_PER
    with tile.TileContext(nc) as tc:
        with tc.tile_pool(name="consts", bufs=1) as consts, \
             tc.tile_pool(name="xs", bufs=4) as xs, \
             tc.tile_pool(name="pr", bufs=2) as pr, \
             tc.tile_pool(name="tr", bufs=2) as tr, \
             tc.tile_pool(name="ep", bufs=1) as ep:
            idx_sb = consts.tile([128, TILES, S_ALL], I32, tag="idx")
            nc.sync.dma_start(out=idx_sb[:], in_=idx[:])
            ebf = consts.tile([128, TILES, D], BF16, tag="ebf")
            nc.sync.dma_start(
                out=ebf[:], in_=esh.rearrange("(p t) d -> p t d", p=128))
            dots = ep.tile([128, TILES, S_ALL], F32, tag="dots")

            for g in range(G):
                t0 = g * T_PER
                samp = xs.tile([128, T_PER * S_ALL, D], BF16, tag="s")
                # HW vector-indirect DMA supports exactly one offset per
                # partition, so issue one gather per (tile, sample).
                for ti in range(T_PER):
                    for s in range(S_ALL):
                        nc.gpsimd.indirect_dma_start(
                            out=samp[:, ti * S_ALL + s, :],
                            out_offset=None,
                            in_=comb[:, :],
                            in_offset=bass.IndirectOffsetOnAxis(
                                ap=idx_sb[:, t0 + ti, s:s + 1], axis=0),
                        )
                prod = pr.tile([128, T_PER, S_ALL, D], BF16, tag="p")
                nc.vector.tensor_tensor(
                    out=prod[:],
                    in0=samp[:].rearrange("p (t s) d -> p t s d", t=T_PER),
                    in1=ebf[:, t0:t0 + T_PER, :]
                        .rearrange("p t (o d) -> p t o d", o=1)
                        .to_broadcast([128, T_PER, S_ALL, D]),
                    op=OP.mult)
                h = tr.tile([128, T_PER, S_ALL, 64], BF16, tag="h")
                nc.vector.tensor_tensor(
                    out=h[:], in0=prod[:, :, :, 0:64],
                    in1=prod[:, :, :, 64:128], op=OP.add)
                q = tr.tile([128, T_PER, S_ALL, 32], BF16, tag="q")
                nc.vector.tensor_tensor(
                    out=q[:], in0=h[:, :, :, 0:32], in1=h[:, :, :, 32:64],
                    op=OP.add)
                e8 = tr.tile([128, T_PER, S_ALL, 16], BF16, tag="e8")
                nc.vector.tensor_tensor(
                    out=e8[:], in0=q[:, :, :, 0:16], in1=q[:, :, :, 16:32],
                    op=OP.add)
                nc.vector.tensor_reduce(
                    out=dots[:, t0:t0 + T_PER, :], in_=e8[:], axis=AX.X,
                    op=OP.add)
                if (g + 1) % BAR_EVERY_G == 0:
                    tc.no_sync_barrier()

            # batched logsumexp epilogue over all tiles
            mx = ep.tile([128, TILES], F32, tag="mx")
            nc.vector.tensor_reduce(out=mx[:], in_=dots[:], axis=AX.X,
                                    op=OP.max)
            subx = ep.tile([128, TILES, S_ALL], F32, tag="subx")
            nc.vector.tensor_tensor(
                out=subx[:], in0=dots[:],
                in1=mx[:].rearrange("p (t o) -> p t o", o=1)
                    .to_broadcast([128, TILES, S_ALL]),
                op=OP.subtract)
            expd = ep.tile([128, TILES, S_ALL], F32, tag="expd")
            nc.scalar.activation(out=expd[:], in_=subx[:], func=AF.Exp,
                                 scale=1.0 / TAU)
            se = ep.tile([128, TILES], F32, tag="se")
            nc.vector.tensor_reduce(out=se[:], in_=expd[:], axis=AX.X,
                                    op=OP.add)
            lnv = ep.tile([128, TILES], F32, tag="lnv")
            nc.scalar.activation(out=lnv[:], in_=se[:], func=AF.Ln)
            t1 = ep.tile([128, TILES], F32, tag="t1")
            nc.vector.tensor_tensor(out=t1[:], in0=mx[:], in1=dots[:, :, 0],
                                    op=OP.subtract)
            scores_sb = ep.tile([128, TILES], F32, tag="scores")
            nc.vector.scalar_tensor_tensor(
                out=scores_sb[:], in0=t1[:], scalar=1.0 / TAU, in1=lnv[:],
                op0=OP.mult, op1=OP.add)
            nc.sync.dma_start(out=scores_out[:], in_=scores_sb[:])
    nc.compile()
    return nc


class _Runner:
    """Compile-once SPMD runner over axon PJRT (8 cores, shard_map)."""

    def __init__(self, nc):
        import jax
        import numpy as np
        from jax.sharding import Mesh, PartitionSpec
        from jax.experimental.shard_map import shard_map
        from concourse import bass2jax, mybir
        from concourse.bass2jax import _bass_exec_p, install_neuronx_cc_hook

        install_neuronx_cc_hook()
        self.jax = jax
        partition_name = (
            nc.partition_id_tensor.name if nc.partition_id_tensor else None
        )
        in_names, out_names, out_avals, zero_outs = [], [], [], []
        for alloc in nc.m.functions[0].allocations:
            if not isinstance(alloc, mybir.MemoryLocationSet):
                continue
            name = alloc.memorylocations[0].name
            if alloc.kind == "ExternalInput":
                if name != partition_name:
                    in_names.append(name)
            elif alloc.kind == "ExternalOutput":
                shape = tuple(alloc.tensor_shape)
                dtype = mybir.dt.np(alloc.dtype)
                out_names.append(name)
                out_avals.append(jax.core.ShapedArray(shape, dtype))
                zero_outs.append(np.zeros(shape, dtype))
        self.in_names, self.out_names = in_names, out_names
        self.out_avals, self.zero_outs = out_avals, zero_outs
        n_params, n_outs = len(in_names), len(out_names)
        all_in_names = in_names + out_names
        if partition_name is not None:
            all_in_names.append(partition_name)
        donate = (tuple(range(n_params, n_params + n_outs))
                  if _DONATE_OUTPUTS else ())

        def _body(*args):
            operands = list(args)
            if partition_name is not None:
                operands.append(bass2jax.partition_id_tensor())
            outs = _bass_exec_p.bind(
                *operands,
                out_avals=tuple(out_avals),
                in_names=tuple(all_in_names),
                out_names=tuple(out_names),
                lowering_input_output_aliases=(),
                sim_require_finite=True,
                sim_require_nnan=True,
                nc=nc,
            )
            return tuple(outs)

        devices = jax.devices()[:N_CORES]
        self.mesh = Mesh(np.asarray(devices), ("core",))
        in_specs = (PartitionSpec("core"),) * (n_params + n_outs)
        out_specs = (PartitionSpec("core"),) * n_outs
        self.fn = jax.jit(
            shard_map(_body, mesh=self.mesh, in_specs=in_specs,
                      out_specs=out_specs, check_rep=False),
            donate_argnums=donate, keep_unused=True)
        self._staged = None

    def stage(self, in_maps):
        from jax.sharding import NamedSharding, PartitionSpec
        concat = [
            np.ascontiguousarray(
                np.concatenate([np.asarray(m[n]) for m in in_maps], axis=0))
            for n in self.in_names
        ]
        sh = NamedSharding(self.mesh, PartitionSpec("core"))
        self._staged = [self.jax.device_put(a, sh) for a in concat]
        self.jax.block_until_ready(self._staged)

    def run(self):
        zeros = [
            np.zeros((N_CORES * z.shape[0], *z.shape[1:]), z.dtype)
            for z in self.zero_outs
        ]
        out = self.fn(*self._staged, *zeros)
        self.jax.block_until_ready(out)
        return [
            {
                n: np.asarray(out[i]).reshape(
                    N_CORES, *self.out_avals[i].shape)[c]
                for i, n in enumerate(self.out_names)
            }
            for c in range(N_CORES)
        ]


def _prep_inputs(embeddings, negative_embs, community_pos_options, neg_idx,
                 iter_n):
    from concourse import mybir
    bf16 = mybir.dt.np(mybir.dt.bfloat16)
    embeddings = np.asarray(embeddings, dtype=np.float32)
    negative_embs = np.asarray(negative_embs, dtype=np.float32)
    comb = np.ascontiguousarray(
        np.concatenate([embeddings, negative_embs], axis=0).astype(bf16))
    cpo = np.asarray(community_pos_options)
    nidx = np.asarray(neg_idx)
    it = int(np.asarray(iter_n))
    pos_idx = cpo[:, it - 1].astype(np.int32)          # [N] rows of comb
    nidx32 = (nidx.astype(np.int32) + np.int32(N))     # [NUM_NEG, N] rows

    in_maps = []
    for c in range(N_CORES):
        base = c * N_SHARD
        esh = np.zeros((N_PAD, D), bf16)
        esh[:N_SHARD] = embeddings[base:base + N_SHARD].astype(bf16)
        pos_pad = np.zeros(N_PAD, np.int32)
        pos_pad[:N_SHARD] = pos_idx[base:base + N_SHARD]
        neg_pad = np.zeros((NUM_NEG, N_PAD), np.int32)
        neg_pad[:, :N_SHARD] = nidx32[:, base:base + N_SHARD]
        # node j = p*196 + t  ->  idx[p, t, s]
        idx = np.empty((128, TILES, S_ALL), np.int32)
        idx[:, :, 0] = pos_pad.reshape(128, TILES)
        idx[:, :, 1:] = neg_pad.reshape(NUM_NEG, 128, TILES).transpose(1, 2, 0)
        in_maps.append({
            "comb": comb,
            "esh": esh,
            "idx": idx,
        })
    return in_maps


def _get_runner():
    global _COMPILED
    if _COMPILED is None:
        _COMPILED = _Runner(_build_nc())
    return _COMPILED


def kernel(embeddings, negative_embs, community_pos_options, neg_idx, iter_n):
    r = _get_runner()
    in_maps = _prep_inputs(embeddings, negative_embs, community_pos_options,
                           neg_idx, iter_n)
    r.stage(in_maps)
    res = r.run()
    total = 0.0
    for c in range(N_CORES):
        sc = res[c]["scores"]         # [128, TILES], node j at [j//196, j%196]
        flat = sc.reshape(-1)         # node-ordered
        total += float(flat[:N_SHARD].astype(np.float64).sum())
    return np.float32(total / N)


# revision 11
# speedup vs baseline: 2.1043x; 2.0745x over previous
"""Trainium2 Bass kernel for nn_ContrastiveLoss (InfoNCE-style loss).

Sharding: data-parallel over nodes N=200000 across 8 NeuronCores
(25000 nodes/core, padded to 25088 = 128 partitions x 196 tile slots).
Node j of a shard maps partition-major: j = p*196 + t, so the shard's
e-rows load as one fully-contiguous HWDGE DMA per core.

Device pipeline per group of T_PER tiles:
  - 11 indirect-DMA row gathers per tile ([128,1] offsets -- the HW
    vector-indirect path supports exactly one offset per partition)
    from a combined [pos;neg] bf16 table, pipelined via a 4-deep pool
  - DVE bf16 multiply (2x perf mode) against the broadcast e-row
  - halving-tree adds in bf16 (+final fp32 tensor_reduce) -> dots
Epilogue (batched over all 196 tiles): rowmax, subtract, ACT Exp,
segmented sum, ACT Ln, score = (m - dots0)/tau + ln(sum).
Host masks padding and takes the global mean over the 8 cores.
"""

import numpy as np

TAU = 0.65
NUM_NEG = 10
S_ALL = NUM_NEG + 1             # pos + 10 negatives
N, M, D = 200000, 200000, 128
N_CORES = 8
N_SHARD = N // N_CORES          # 25000
TILES = 196                     # ceil(25000/128)
N_PAD = TILES * 128             # 25088
T_PER = 2                       # tiles per DVE compute group
BAR_EVERY_G = 2                 # groups between scheduler fences

_COMPILED = None
_DONATE_OUTPUTS = True          # sim_check disables (CPU can't alias)


def _build_nc():
    import concourse.bass as bass
    import concourse.bacc as bacc
    import concourse.tile as tile
    from concourse import mybir

    F32 = mybir.dt.float32
    BF16 = mybir.dt.bfloat16
    I32 = mybir.dt.int32
    AF = mybir.ActivationFunctionType
    OP = mybir.AluOpType
    AX = mybir.AxisListType

    nc = bacc.Bacc("TRN2", target_bir_lowering=False, debug=False,
                   num_devices=N_CORES)
    comb = nc.dram_tensor("comb", [N + M, D], BF16, kind="ExternalInput").ap()
    esh = nc.dram_tensor("esh", [N_PAD, D], BF16, kind="ExternalInput").ap()
    idx = nc.dram_tensor("idx", [128, TILES, S_ALL], I32,
                         kind="ExternalInput").ap()
    scores_out = nc.dram_tensor("scores", [128, TILES], F32,
                                kind="ExternalOutput").ap()

    G = TILES // T_PER
    with tile.TileContext(nc) as tc:
        with tc.tile_pool(name="consts", bufs=1) as consts, \
             tc.tile_pool(name="xs", bufs=4) as xs, \
             tc.tile_pool(name="pr", bufs=2) as pr, \
             tc.tile_pool(name="tr", bufs=2) as tr, \
             tc.tile_pool(name="ep", bufs=1) as ep:
            idx_sb = consts.tile([128, TILES, S_ALL], I32, tag="idx")
            nc.sync.dma_start(out=idx_sb[:], in_=idx[:])
            ebf = consts.tile([128, TILES, D], BF16, tag="ebf")
            nc.sync.dma_start(
                out=ebf[:], in_=esh.rearrange("(p t) d -> p t d", p=128))
            dots = ep.tile([128, TILES, S_ALL], F32, tag="dots")

            for g in range(G):
                t0 = g * T_PER
                samp = xs.tile([128, T_PER * S_ALL, D], BF16, tag="s")
                # HW vector-indirect DMA supports exactly one offset per
                # partition, so issue one gather per (tile, sample).
                for ti in range(T_PER):
                    for s in range(S_ALL):
                        nc.gpsimd.indirect_dma_start(
                            out=samp[:, ti * S_ALL + s, :],
                            out_offset=None,
                            in_=comb[:, :],
                            in_offset=bass.IndirectOffsetOnAxis(
                                ap=idx_sb[:, t0 + ti, s:s + 1], axis=0),
                        )
                prod = pr.tile([128, T_PER, S_ALL, D], BF16, tag="p")
                nc.vector.tensor_tensor(
                    out=prod[:],
                    in0=samp[:].rearrange("p (t s) d -> p t s d", t=T_PER),
                    in1=ebf[:, t0:t0 + T_PER, :]
                        .rearrange("p t (o d) -> p t o d", o=1)
                        .to_broadcast([128, T_PER, S_ALL, D]),
                    op=OP.mult)
                h = tr.tile([128, T_PER, S_ALL, 64], BF16, tag="h")
                nc.vector.tensor_tensor(
                    out=h[:], in0=prod[:, :, :, 0:64],
                    in1=prod[:, :, :, 64:128], op=OP.add)
                q = tr.tile([128, T_PER, S_ALL, 32], BF16, tag="q")
                nc.vector.tensor_tensor(
                    out=q[:], in0=h[:, :, :, 0:32], in1=h[:, :, :, 32:64],
                    op=OP.add)
                e8 = tr.tile([128, T_PER, S_ALL, 16], BF16, tag="e8")
                nc.vector.tensor_tensor(
                    out=e8[:], in0=q[:, :, :, 0:16], in1=q[:, :, :, 16:32],
                    op=OP.add)
                nc.vector.tensor_reduce(
                    out=dots[:, t0:t0 + T_PER, :], in_=e8[:], axis=AX.X,
                    op=OP.add)
                if (g + 1) % BAR_EVERY_G == 0:
                    tc.no_sync_barrier()

            # batched logsumexp epilogue over all tiles
            mx = ep.tile([128, TILES], F32, tag="mx")
            nc.vector.tensor_reduce(out=mx[:], in_=dots[:], axis=AX.X,
                                    op=OP.max)
            subx = ep.tile([128, TILES, S_ALL], F32, tag="subx")
            nc.vector.tensor_tensor(
                out=subx[:], in0=dots[:],
                in1=mx[:].rearrange("p (t o) -> p t o", o=1)
                    .to_broadcast([128, TILES, S_ALL]),
                op=OP.subtract)
            expd = ep.tile([128, TILES, S_ALL], F32, tag="expd")
            nc.scalar.activation(out=expd[:], in_=subx[:], func=AF.Exp,
                                 scale=1.0 / TAU)
            se = ep.tile([128, TILES], F32, tag="se")
            nc.vector.tensor_reduce(out=se[:], in_=expd[:], axis=AX.X,
                                    op=OP.add)
            lnv = ep.tile([128, TILES], F32, tag="lnv")
            nc.scalar.activation(out=lnv[:], in_=se[:], func=AF.Ln)
            t1 = ep.tile([128, TILES], F32, tag="t1")
            nc.vector.tensor_tensor(out=t1[:], in0=mx[:], in1=dots[:, :, 0],
                                    op=OP.subtract)
            scores_sb = ep.tile([128, TILES], F32, tag="scores")
            nc.vector.scalar_tensor_tensor(
                out=scores_sb[:], in0=t1[:], scalar=1.0 / TAU, in1=lnv[:],
                op0=OP.mult, op1=OP.add)
            nc.sync.dma_start(out=scores_out[:], in_=scores_sb[:])
    nc.compile()
    return nc


class _Runner:
    """Compile-once SPMD runner over axon PJRT (8 cores, shard_map)."""

    def __init__(self, nc):
        import jax
        import numpy as np
        from jax.sharding import Mesh, PartitionSpec
        from jax.experimental.shard_map import shard_map
        from concourse import bass2jax, mybir
        from concourse.bass2jax import _bass_exec_p, install_neuronx_cc_hook

        install_neuronx_cc_hook()
        self.jax = jax
        partition_name = (
            nc.partition_id_tensor.name if nc.partition_id_tensor else None
        )
        in_names, out_names, out_avals, zero_outs = [], [], [], []
        for alloc in nc.m.functions[0].allocations:
            if not isinstance(alloc, mybir.MemoryLocationSet):
                continue
            name = alloc.memorylocations[0].name
            if alloc.kind == "ExternalInput":
                if name != partition_name:
                    in_names.append(name)
            elif alloc.kind == "ExternalOutput":
                shape = tuple(alloc.tensor_shape)
                dtype = mybir.dt.np(alloc.dtype)
                out_names.append(name)
                out_avals.append(jax.core.ShapedArray(shape, dtype))
                zero_outs.append(np.zeros(shape, dtype))
        self.in_names, self.out_names = in_names, out_names
        self.out_avals, self.zero_outs = out_avals, zero_outs
        n_params, n_outs = len(in_names), len(out_names)
        all_in_names = in_names + out_names
        if partition_name is not None:
            all_in_names.append(partition_name)
        donate = (tuple(range(n_params, n_params + n_outs))
                  if _DONATE_OUTPUTS else ())

        def _body(*args):
            operands = list(args)
            if partition_name is not None:
                operands.append(bass2jax.partition_id_tensor())
            outs = _bass_exec_p.bind(
                *operands,
                out_avals=tuple(out_avals),
                in_names=tuple(all_in_names),
                out_names=tuple(out_names),
                lowering_input_output_aliases=(),
                sim_require_finite=True,
                sim_require_nnan=True,
                nc=nc,
            )
            return tuple(outs)

        devices = jax.devices()[:N_CORES]
        self.mesh = Mesh(np.asarray(devices), ("core",))
        in_specs = (PartitionSpec("core"),) * (n_params + n_outs)
        out_specs = (PartitionSpec("core"),) * n_outs
        self.fn = jax.jit(
            shard_map(_body, mesh=self.mesh, in_specs=in_specs,
                      out_specs=out_specs, check_rep=False),
            donate_argnums=donate, keep_unused=True)
        self._staged = None

    def stage(self, in_maps):
        from jax.sharding import NamedSharding, PartitionSpec
        concat = [
            np.ascontiguousarray(
                np.concatenate([np.asarray(m[n]) for m in in_maps], axis=0))
            for n in self.in_names
        ]
        sh = NamedSharding(self.mesh, PartitionSpec("core"))
        self._staged = [self.jax.device_put(a, sh) for a in concat]
        self.jax.block_until_ready(self._staged)

    def run(self):
        zeros = [
            np.zeros((N_CORES * z.shape[0], *z.shape[1:]), z.dtype)
            for z in self.zero_outs
        ]
        out = self.fn(*self._staged, *zeros)
        self.jax.block_until_ready(out)
        return [
            {
                n: np.asarray(out[i]).reshape(
                    N_CORES, *self.out_avals[i].shape)[c]
                for i, n in enumerate(self.out_names)
            }
            for c in range(N_CORES)
        ]


def _prep_inputs(embeddings, negative_embs, community_pos_options, neg_idx,
                 iter_n):
    from concourse import mybir
    bf16 = mybir.dt.np(mybir.dt.bfloat16)
    embeddings = np.asarray(embeddings, dtype=np.float32)
    negative_embs = np.asarray(negative_embs, dtype=np.float32)
    comb = np.ascontiguousarray(
        np.concatenate([embeddings, negative_embs], axis=0).astype(bf16))
    cpo = np.asarray(community_pos_options)
    nidx = np.asarray(neg_idx)
    it = int(np.asarray(iter_n))
    pos_idx = cpo[:, it - 1].astype(np.int32)          # [N] rows of comb
    nidx32 = (nidx.astype(np.int32) + np.int32(N))     # [NUM_NEG, N] rows

    in_maps = []
    for c in range(N_CORES):
        base = c * N_SHARD
        esh = np.zeros((N_PAD, D), bf16)
        esh[:N_SHARD] = embeddings[base:base + N_SHARD].astype(bf16)
        pos_pad = np.zeros(N_PAD, np.int32)
        pos_pad[:N_SHARD] = pos_idx[base:base + N_SHARD]
        neg_pad = np.zeros((NUM_NEG, N_PAD), np.int32)
        neg_pad[:, :N_SHARD] = nidx32[:, base:base + N_SHARD]
        # node j = p*196 + t  ->  idx[p, t, s]
        idx = np.empty((128, TILES, S_ALL), np.int32)
        idx[:, :, 0] = pos_pad.reshape(128, TILES)
        idx[:, :, 1:] = neg_pad.reshape(NUM_NEG, 128, TILES).transpose(1, 2, 0)
        in_maps.append({
            "comb": comb,
            "esh": esh,
            "idx": idx,
        })
    return in_maps


def _get_runner():
    global _COMPILED
    if _COMPILED is None:
        _COMPILED = _Runner(_build_nc())
    return _COMPILED


def kernel(embeddings, negative_embs, community_pos_options, neg_idx, iter_n):
    r = _get_runner()
    in_maps = _prep_inputs(embeddings, negative_embs, community_pos_options,
                           neg_idx, iter_n)
    r.stage(in_maps)
    res = r.run()
    total = 0.0
    for c in range(N_CORES):
        sc = res[c]["scores"]         # [128, TILES], node j at [j//196, j%196]
        flat = sc.reshape(-1)         # node-ordered
        total += float(flat[:N_SHARD].astype(np.float64).sum())
    return np.float32(total / N)
